# revision 19
# baseline (speedup 1.0000x reference)
"""StyleGAN2-style modulated 3x3 conv (B=16, C=128, H=W=128) on 8 TRN2 NeuronCores.

Sharding: data-parallel over batch (2 samples/core). The grouped conv runs as 9
accumulated bf16 matmuls per 4-row PSUM tile with the input-channel dim (128) as
the contraction. bf16 streams at the PE's 1 elem/cycle floor (~213ns/512 rows);
fp32r pays an extra high-half stationary self-load (~282ns) and mixed
bf16-weight/fp32r-moving is rejected by walrus (NCC_IBIR034), so both operands
are bf16. The host pre-casts x/weight to bf16 and pre-transposes the weight to
W_t[i, (kh kw o)] layout (input marshalling only — style modulation,
demodulation and the conv itself all run on device).

The mod/demod normalizers (style/max|style|, w/(sqrt(ikk)*max|w|)) cancel
exactly through the demodulation: out = conv(x, w*s) * rsqrt(sum((w*s)^2)+eps'),
where eps' differs from the reference's eps by a ~1e-5-relative rescale. So the
device modulates RAW weights with RAW style and demodulates with a
rsqrt-of-matvec of raw squares — no max-reductions on the critical path.

Both samples' images live whole in SBUF ([C, 128, 130] bf16 with zero pad
columns); input DMA is chunked on the sync hardware queue in consumption order
(the gpsimd software DGE measures ~78 GB/s — too slow for bulk loads). Output
is stored as bf16 (halves store traffic; host casts back to fp32). Drains
alternate scalar/vector engines.
"""

from itertools import product

import ml_dtypes
import numpy as np

import concourse.bacc as bacc
import concourse.bass as bass
import concourse.mybir as mybir
import concourse.tile as tile
from concourse.bass_utils import run_bass_kernel_spmd
from concourse.masks import make_identity

B, C, H, W = 16, 128, 128, 128
KK = 3
EPS = 1e-8
N_CORES = 8
S = B // N_CORES          # samples per core
RPT = 4                   # output rows per PSUM tile (one PSUM bank)
WP = W + 2                # image width incl. 1-col zero pad each side
GT = 4                    # PSUM tiles per output store DMA
NKK = KK * KK

FP32 = mybir.dt.float32
BF16 = mybir.dt.bfloat16


def build_bass() -> bass.Bass:
    nc = bacc.Bacc(None)
    # x arrives host-padded to W+2 with zero columns: per-partition rows are
    # then contiguous 260B in both DRAM and SBUF, so each chunk moves as one
    # max-size descriptor per partition (256B-element DMAs measured ~150GB/s
    # and hogged the queue for ~30us).
    x_d = nc.dram_tensor("x", [S, C, H, WP], BF16, kind="ExternalInput")
    wt_d = nc.dram_tensor("wt", [C, NKK * C], BF16, kind="ExternalInput")
    style_d = nc.dram_tensor("style", [S, C], FP32, kind="ExternalInput")
    out_d = nc.dram_tensor("out", [S, C, H, W], BF16, kind="ExternalOutput")

    with tile.TileContext(nc) as tc:
        with (
            tc.tile_pool(name="const", bufs=1) as const_pool,
            tc.tile_pool(name="wpool", bufs=1) as wpool,
            tc.tile_pool(name="xpool", bufs=1) as xpool,
            tc.tile_pool(name="opool", bufs=4) as opool,
            tc.tile_pool(name="psum_conv", bufs=6, space="PSUM") as psum_conv,
            tc.tile_pool(name="psum_misc", bufs=1, space="PSUM") as psum_misc,
        ):
            # ---- engine warmups ----
            # junk bf16 data, ready immediately: 3 x 512-free matmuls walk the
            # PE out of its low/mid p-state before the real stream arrives,
            # and one vector op absorbs the DVE's ~2.1us cold-start.
            junk = const_pool.tile([128, 512], BF16)
            nc.gpsimd.memset(junk[:], 0.0)
            ident = const_pool.tile([128, 128], FP32)
            make_identity(nc, ident)
            ps_warm = psum_conv.tile([C, 512], FP32, name="warm", tag="ps")
            for wi in range(3):
                nc.tensor.matmul(
                    ps_warm[:], junk[:, 0:128], junk[:],
                    start=(wi == 0), stop=(wi == 2), skip_group_check=True,
                )
            vwarm = const_pool.tile([128, 128], BF16)
            nc.vector.tensor_scalar_mul(vwarm[:], junk[:, 0:128], 0.0)

            # ---- input DMAs ----
            # style + weights ride the scalar-engine DGE so their completion
            # never queues behind the multi-MB image transfers on sync; the
            # images are chunked so arrival outruns the PE's ~2 rows/us
            # consumption.
            srow = wpool.tile([S, C], FP32)
            nc.scalar.dma_start(srow[:], style_d[:])
            W_t = wpool.tile([C, NKK * C], BF16)
            nc.scalar.dma_start(W_t[:], wt_d[:])

            xs = [
                xpool.tile([C, H, WP], BF16, name=f"xs{b}", tag=f"xs{b}")
                for b in range(S)
            ]
            for ja, jb in ((0, 6), (6, 16), (16, 40), (40, H)):
                nc.sync.dma_start(xs[0][:, ja:jb, :], x_d[0, :, ja:jb, :])
            nc.sync.dma_start(xs[1][:], x_d[1, :, :, :])

            # ---- style path: transpose RAW style to columns; the
            # normalizers cancel through the demodulation ----
            scol = wpool.tile([C, 2 * S], FP32)
            pt_s = psum_misc.tile([C, S], FP32, name="pts", tag="pts")
            nc.tensor.transpose(pt_s[:], srow[:], ident[0:S, 0:S])
            nc.scalar.activation(
                scol[:, 0:S], pt_s[:], mybir.ActivationFunctionType.Copy,
            )

            # taps ordered so (1,1) — always full-size, no pad columns —
            # comes first: it is the start=True matmul of every accumulation
            # group and only needs W_t + the first image rows + scol.
            TAPS = [(1, 1)] + [t for t in product(range(KK), range(KK)) if t != (1, 1)]

            # per-sample modulated weights: wmod[b][i, (k o)] = W_t * s[b, i]
            wmod = [
                wpool.tile([C, NKK * C], BF16, name=f"wmod{b}", tag=f"wmod{b}")
                for b in range(S)
            ]
            # modulate sample-0 taps split across vector+scalar so the tap
            # stream outruns the PE's ~218ns/tap consumption; everything not
            # needed until coe (~12us) or sample 1 (~75us) is emitted after.
            for ti, (dy, dx) in enumerate(TAPS):
                k = dy * KK + dx
                if ti not in (5, 7, 8):
                    # vector: ~240ns/tap, outruns the PE's 218ns/tap
                    nc.vector.tensor_scalar_mul(
                        wmod[0][:, k * C:(k + 1) * C],
                        W_t[:, k * C:(k + 1) * C], scol[:, 0:1],
                    )
                else:
                    # scalar: ~480ns/tap, gets the late-needed taps
                    nc.scalar.activation(
                        wmod[0][:, k * C:(k + 1) * C],
                        W_t[:, k * C:(k + 1) * C],
                        mybir.ActivationFunctionType.Copy,
                        bias=0.0, scale=scol[:, 0:1],
                    )
            nc.vector.tensor_mul(scol[:, S:2 * S], scol[:, 0:S], scol[:, 0:S])
            # wsq / qt for the demod matvec (gpsimd is otherwise idle)
            wsq = wpool.tile([C, NKK * C], FP32)
            nc.gpsimd.tensor_mul(wsq[:], W_t[:], W_t[:])
            qt = wpool.tile([C, C], FP32)
            nc.vector.tensor_reduce(
                qt[:], wsq[:].rearrange("i (k o) -> i o k", k=NKK),
                axis=mybir.AxisListType.X, op=mybir.AluOpType.add,
            )
            # sample-1 modulated weights (needed only much later)
            nc.vector.tensor_scalar_mul(wmod[1][:], W_t[:], scol[:, 1:2])
            eps_tile = wpool.tile([C, 1], FP32)
            nc.gpsimd.memset(eps_tile[:], EPS)
            coe = wpool.tile([C, S], FP32)

            NT = H // RPT  # PSUM tiles per sample
            coe_emitted = False
            for b in range(S):
                for g in range(NT // GT):
                    gy = g * GT * RPT
                    ot = opool.tile([C, GT * RPT, W], BF16, name="ot", tag="ot")
                    deferred = []
                    for u in range(GT):
                        yl = gy + u * RPT
                        ps = psum_conv.tile([C, RPT * W], FP32, name="ps", tag="ps")
                        for idx, (dy, dx) in enumerate(TAPS):
                            ra = 1 if (yl == 0 and dy == 0) else 0
                            rb = RPT - 1 if (yl == H - RPT and dy == 2) else RPT
                            nc.tensor.matmul(
                                ps[:, ra * W:rb * W],
                                wmod[b][:, (dy * KK + dx) * C:(dy * KK + dx + 1) * C],
                                xs[b][:, yl + dy - 1 + ra:yl + dy - 1 + rb, dx:dx + W],
                                start=(idx == 0),
                                stop=(idx == NKK - 1),
                                skip_group_check=True,
                            )
                        deferred.append((u, ps))
                        if not coe_emitted:
                            if u < 2:
                                # defer early drains until coe exists
                                continue
                            # demod scale: coe[o,b] = rsqrt(Q[o,b] + eps),
                            # Q = qt^T @ s^2. Emitted after the third tile's
                            # matmuls but before the first drains, which
                            # read coe.
                            coe_emitted = True
                            ps_coe = psum_misc.tile([C, S], FP32, tag="ps_coe")
                            nc.tensor.matmul(
                                ps_coe[:], qt[:], scol[:, S:2 * S],
                                start=True, stop=True,
                            )
                            nc.scalar.activation(
                                coe[:], ps_coe[:], mybir.ActivationFunctionType.Sqrt,
                                bias=eps_tile[:], scale=1.0,
                            )
                            nc.vector.reciprocal(coe[:], coe[:])
                        last_group = (b == S - 1) and (g == NT // GT - 1)
                        for ud, psd in deferred:
                            ots = ot[:, ud * RPT:(ud + 1) * RPT, :]
                            ps_r = psd[:].rearrange("c (r w) -> c r w", r=RPT)
                            if ud % 2 == 0:
                                nc.scalar.activation(
                                    ots, ps_r, mybir.ActivationFunctionType.Copy,
                                    bias=0.0, scale=coe[:, b:b + 1],
                                )
                            else:
                                nc.vector.tensor_scalar_mul(
                                    ots, ps_r, coe[:, b:b + 1],
                                )
                            if last_group:
                                nc.sync.dma_start(
                                    out_d[b, :, gy + ud * RPT:gy + (ud + 1) * RPT, :],
                                    ots,
                                )
                        deferred = []
                    if not last_group:
                        nc.sync.dma_start(
                            out_d[b, :, gy:gy + GT * RPT, :], ot[:],
                        )

    nc.compile()
    return nc


_CACHED = {}


def kernel(x: np.ndarray, style: np.ndarray, weight: np.ndarray, trace: bool = False):
    # input marshalling (host): bf16 precision of the conv operands is a
    # kernel design decision (rel err ~2.8e-3); the weight transpose to
    # W_t[i, (kh kw o)] is a pure layout transform. All module math (style
    # modulation, demodulation, conv) runs on device.
    xb = np.zeros((B, C, H, WP), dtype=ml_dtypes.bfloat16)
    xb[..., 1:W + 1] = x
    style = np.ascontiguousarray(style, dtype=np.float32)
    wt = np.ascontiguousarray(
        weight.astype(np.float32).transpose(1, 2, 3, 0).reshape(C, NKK * C)
    ).astype(ml_dtypes.bfloat16)

    if "nc" not in _CACHED:
        _CACHED["nc"] = build_bass()
    nc = _CACHED["nc"]

    in_maps = [
        {
            "x": xb[i * S:(i + 1) * S],
            "wt": wt,
            "style": style[i * S:(i + 1) * S],
        }
        for i in range(N_CORES)
    ]
    res = run_bass_kernel_spmd(
        nc, in_maps, core_ids=list(range(N_CORES)), trace=trace,
    )
    out = np.concatenate(
        [r["out"].astype(np.float32) for r in res.results], axis=0
    )
    if trace:
        kernel.last_results = res
    return out
